# revision 28
# baseline (speedup 1.0000x reference)
"""Distributed Trainium2 kernel for nn_ACTLoss_56624848831010.

Math note (exact simplification of the reference):
  losses_per_step[k, b] = ce[b] + k * 0.01 is strictly increasing in k, so
  optimal_k == 0 for every sample regardless of logits/labels.  With
  update_critic == 0 the loss therefore reduces to

      mask  = halt > 0
      s[b]  = sum_{j < halt[b]} contributions[j, b]          (cumsum select)
      per[b]= -0.1 * halt[b] * log(s[b] / max(halt[b],1) + 1e-8)
      loss  = sum(per * mask) / max(sum(mask), 1)   (0 if no mask)

  (per is exactly 0 whenever halt == 0, so the sum needs no extra mask.)
  logits / labels / thresholds never influence the output; with
  update_critic != 0 the mask (0 < optimal_k <= K) is identically false and
  the loss is exactly 0.0.

Distribution note: a data-parallel shard + psum(sum, count) was implemented
and measured first, but in this environment a single 32-byte AllGather has a
~65 us latency floor (launch skew across the 8 PJRT-dispatched cores), which
dominated everything else (86 us total vs ~33 us for this version).
Collective-free plan: every core redundantly computes the full 32768-sample
reduction on-device (2 MB of contributions + halt per core, ~3 DVE passes
over 16x32768 plus O(B) tail ops) with zero cross-core communication.
Measured ~33-37 us end-to-end (run-to-run DVFS variance), of which ~10 us is
NEFF preamble + first-DMA latency and ~4 us kernel-tail drain.
"""

import numpy as np

_B = 32768
_K = 16
_M = 8  # cores
_P = 128
_C = _B // _P  # 256 samples per partition (full batch on every core)
_CSZ = (8, 4, 2, 2)  # j-chunk sizes: big early chunks, tiny late ones
_COF = (0, 8, 12, 14)  # chunk offsets

_CACHED = None
LAST_RESULTS = None  # BassKernelResults of the last run (for test harness)


def _build_nc():
    import concourse.mybir as mybir
    from concourse import bacc, tile

    f32 = mybir.dt.float32
    bf16 = mybir.dt.bfloat16
    i32 = mybir.dt.int32
    Alu = mybir.AluOpType
    Act = mybir.ActivationFunctionType
    Ax = mybir.AxisListType

    nc = bacc.Bacc(None, target_bir_lowering=False, num_devices=_M)

    cont = nc.declare_dram_parameter("contributions", [_K, _B], f32, isOutput=False)
    halt = nc.declare_dram_parameter("halt", [_B], i32, isOutput=False)
    out = nc.declare_dram_parameter("out", [1, 1], f32, isOutput=True)

    with tile.TileContext(nc) as tc:
        with (
            tc.tile_pool(name="sb", bufs=1) as sb,
            tc.tile_pool(name="ps", bufs=1, space="PSUM") as ps,
        ):
            hi = sb.tile([_P, _C], i32)
            kph = sb.tile([_P, _C], bf16)
            cts = [sb.tile([_P, _CSZ[i], _C], f32, name=f"ct{i}", tag=f"ct{i}") for i in range(4)]
            inds = [sb.tile([_P, _CSZ[i], _C], bf16, name=f"ind{i}", tag=f"ind{i}") for i in range(4)]
            u = sb.tile([_P, _C], f32)
            v = sb.tile([_P, _C], f32)
            lnt = sb.tile([_P, _C], f32)
            lnk = sb.tile([_P, _C], f32)
            lp = sb.tile([_P, _C], f32)
            pv = sb.tile([_P, _C], f32)
            red = sb.tile([_P, 2], f32)
            ones = sb.tile([_P, 1], f32)
            z0 = sb.tile([_P, 1], f32)
            fin = sb.tile([1, 4], f32)
            ps1 = ps.tile([1, 2], f32)

            # --- loads: halt first (kpf feeds everything), contributions by
            # j-chunks so masking can start while later chunks stream in ---
            nc.scalar.dma_start(
                out=hi[:], in_=halt[:].rearrange("(p c) -> p c", p=_P)
            )
            for i in range(4):
                eng = nc.sync if i % 2 == 0 else nc.scalar
                j0, j1 = _COF[i], _COF[i] + _CSZ[i]
                eng.dma_start(
                    out=cts[i][:],
                    in_=cont[j0:j1, :].rearrange("j (p c) -> p j c", p=_P),
                )

            # constants; the early dummy Ln preloads the ACT table
            # concurrently with the DMAs instead of on the critical path
            nc.vector.memset(ones[:], 1.0)
            nc.vector.memset(z0[:], 0.0)
            nc.scalar.activation(
                out=fin[0:1, 0:1], in_=ones[0:1, :], func=Act.Ln, bias=z0[0:1, :]
            )
            # kp in bf16 (exact for 0..16); drives the per-j compares in
            # the DVE 4x tensor_scalar mode and the later u / pv ops
            nc.vector.tensor_copy(out=kph[:], in_=hi[:])

            # ind[p, j, c] = (kp[p, c] > j): dense bf16 tensor_scalar per j
            # (4x mode).  All 16 run before the first mask multiply so the
            # DVE stream never stalls behind them mid-pipeline.
            for i in range(4):
                for j in range(_COF[i], _COF[i] + _CSZ[i]):
                    nc.vector.tensor_scalar(
                        out=inds[i][:, j - _COF[i], :], in0=kph[:],
                        scalar1=float(j), scalar2=None, op0=Alu.is_gt,
                    )

            # per-chunk mask multiply + intra-chunk reduce [P,4,C] -> [P,1,C],
            # emitted in DMA arrival order so the static DVE stream only has a
            # short tail after the last chunk lands.  Gap-filler ops (u, red1)
            # slot between chunk 0 and chunk 1.
            def intra(i):
                w = _CSZ[i]
                while w > 1:
                    h = w // 2
                    nc.vector.tensor_tensor(
                        out=cts[i][:, 0:h, :], in0=cts[i][:, 0:h, :],
                        in1=cts[i][:, h:w, :], op=Alu.add,
                    )
                    w = h

            nc.vector.tensor_tensor(
                out=cts[0][:], in0=cts[0][:], in1=inds[0][:], op=Alu.mult
            )
            intra(0)
            # u = max(kp, 1) * 1e-8  (fused two-op tensor_scalar); feeds the
            # ScalarE Ln(u*1e8) = ln(max(kp,1)) which runs in parallel
            nc.vector.tensor_scalar(
                out=u[:], in0=kph[:], scalar1=1.0, scalar2=1e-8, op0=Alu.max,
                op1=Alu.mult,
            )
            nc.scalar.activation(
                out=lnk[:], in_=u[:], func=Act.Ln, bias=z0[:], scale=1e8
            )
            # mask count: red[:,1] = sum_c (kp > 0)
            nc.vector.tensor_reduce(
                out=red[:, 1:2], in_=inds[0][:, 0, :], axis=Ax.X, op=Alu.add
            )
            nc.vector.tensor_tensor(
                out=cts[1][:], in0=cts[1][:], in1=inds[1][:], op=Alu.mult
            )
            intra(1)
            nc.vector.tensor_tensor(
                out=cts[0][:, 0:1, :], in0=cts[0][:, 0:1, :],
                in1=cts[1][:, 0:1, :], op=Alu.add,
            )
            nc.vector.tensor_tensor(
                out=cts[2][:], in0=cts[2][:], in1=inds[2][:], op=Alu.mult
            )
            intra(2)
            nc.vector.tensor_tensor(
                out=cts[3][:], in0=cts[3][:], in1=inds[3][:], op=Alu.mult
            )
            intra(3)
            nc.vector.tensor_tensor(
                out=cts[2][:, 0:1, :], in0=cts[2][:, 0:1, :],
                in1=cts[3][:, 0:1, :], op=Alu.add,
            )
            nc.vector.tensor_tensor(
                out=cts[0][:, 0:1, :], in0=cts[0][:, 0:1, :],
                in1=cts[2][:, 0:1, :], op=Alu.add,
            )

            # lp = ln(s/max(kp,1) + 1e-8) = ln(s + u) - ln(max(kp,1))
            nc.vector.tensor_tensor(
                out=v[:], in0=cts[0][:, 0, :], in1=u[:], op=Alu.add
            )
            nc.scalar.activation(out=lnt[:], in_=v[:], func=Act.Ln, bias=z0[:])
            nc.vector.tensor_tensor(out=lp[:], in0=lnt[:], in1=lnk[:], op=Alu.subtract)

            # red[:,0] = sum_c kp*lp
            nc.vector.tensor_tensor(out=pv[:], in0=lp[:], in1=kph[:], op=Alu.mult)
            nc.vector.tensor_reduce(out=red[:, 0:1], in_=pv[:], axis=Ax.X, op=Alu.add)

            # partition reduce -> (sum, cnt), then the masked mean
            nc.tensor.matmul(ps1[:], ones[:], red[:], start=True, stop=True)
            nc.vector.tensor_scalar(
                out=fin[0:1, 0:1], in0=ps1[0:1, 1:2], scalar1=1.0, scalar2=None,
                op0=Alu.max,
            )
            nc.vector.reciprocal(out=fin[0:1, 2:3], in_=fin[0:1, 0:1])
            nc.vector.tensor_tensor(
                out=fin[0:1, 3:4], in0=ps1[0:1, 0:1], in1=fin[0:1, 2:3], op=Alu.mult
            )
            nc.vector.tensor_scalar(
                out=fin[0:1, 3:4], in0=fin[0:1, 3:4], scalar1=-0.1, scalar2=None,
                op0=Alu.mult,
            )
            nc.sync.dma_start(out=out[:], in_=fin[0:1, 3:4])

    nc.compile()
    return nc


def kernel(
    logits=None,
    labels=None,
    contributions=None,
    thresholds=None,
    halt_iterations=None,
    update_critic=0,
    **_unused,
):
    global _CACHED, LAST_RESULTS

    if int(np.asarray(update_critic)) != 0:
        # optimal_k == 0 makes the critic mask (0 < k <= K) identically false.
        return np.zeros((), dtype=np.float32)

    cont = np.ascontiguousarray(np.asarray(contributions, dtype=np.float32))
    halt = np.ascontiguousarray(np.asarray(halt_iterations).astype(np.int32))
    assert cont.shape == (_K, _B) and halt.shape == (_B,)

    if _CACHED is None:
        _CACHED = _build_nc()
    nc = _CACHED

    from concourse.bass_utils import run_bass_kernel_spmd

    in_maps = [{"contributions": cont, "halt": halt} for _ in range(_M)]
    # the axon-proxied device occasionally reports a transient
    # NRT_EXEC_UNIT_UNRECOVERABLE; it recovers on the next attempt
    last_err = None
    for _attempt in range(3):
        try:
            res = run_bass_kernel_spmd(nc, in_maps, core_ids=list(range(_M)))
            break
        except Exception as e:  # noqa: BLE001
            last_err = e
            import time

            time.sleep(2.0)
    else:
        raise last_err
    LAST_RESULTS = res
    return np.asarray(res.results[0]["out"], dtype=np.float32).reshape(())


if __name__ == "__main__":
    rng = np.random.default_rng(0)
    c = rng.random((_K, _B), dtype=np.float32)
    h = rng.integers(0, _K + 1, size=(_B,)).astype(np.int64)
    outv = kernel(contributions=c, halt_iterations=h)
    cum = np.cumsum(c, axis=0)
    idx = np.clip(h - 1, 0, _K - 1)
    s = cum[idx, np.arange(_B)]
    kpm = np.maximum(h, 1).astype(np.float32)
    per = 0.1 * h.astype(np.float32) * np.log(s / kpm + 1e-8) * -1.0
    m = h > 0
    ref = (per * m).sum() / max(m.sum(), 1)
    print("kernel:", outv, "ref:", ref, "relerr:", abs(outv - ref) / abs(ref))
